# revision 20
# baseline (speedup 1.0000x reference)
"""Trainium2 Bass kernel: BackwardInjectDepthwiseConv2D (B=32,H=W=56,C=256,K=3).

Outputs (grad_in, grad_wt, grad_bias) for a depthwise conv backward:
  grad_in[b,y,x,c]  = sum_{i,j} g_pad[b,y+i,x+j,c] * kernels[2-i,2-j,c,0]
  grad_wt[i,j,c]    = sum_{b,y,x} x_pad[b,y+i,x+j,c] * g_out[b,y,x,c]
  grad_bias[c]      = sum_{b,y,x} g_out[b,y,x,c]

Strategy (8 NeuronCores, pure batch data-parallel, 4 images/core):
 - Host reshapes to channels-on-partitions layout [img, c, 58*58] (zero
   padded borders) in bf16; per core 8 "pairs" = 4 images x 2 channel
   halves of 128.
 - grad_in on TensorE: 9 accumulating diag-matmuls per pair (lhsT =
   diag(w_tap) per channel-half); shifted taps are free-dim slices of
   the padded tile.  PSUM fp32 accumulation, ScalarE evacuates.
 - grad_wt on VectorE: per tap a bf16 tensor_tensor multiply (2x mode)
   feeding a per-partition free-dim sum, split between VectorE
   (tensor_scalar accum_out, 4x mode) and ScalarE (activation
   accum_out) to balance engines.
 - grad_bias rides the same reduce path on g.
 - Weight/bias grads are per-core partials summed on host.
"""

import sys

for _p in ("/opt/trn_rl_repo",):
    if _p not in sys.path:
        sys.path.insert(0, _p)

import numpy as np
import ml_dtypes
from contextlib import ExitStack

B, H, W, C, K = 32, 56, 56, 256, 3
NCORES = 8
IMG_PER_CORE = B // NCORES          # 4
NPAIR = IMG_PER_CORE * 2            # 4 images x 2 channel halves
HP = H + 2                          # 58 (padded rows)
WP = W + 2                          # 58 (padded cols)
S_PAD = HP * WP                     # 3364
S_TILE = 3368                       # tile free size (tail pad, even)
SV = H * WP                         # 3248: output window (56 rows of 58)
OFF_C = WP + 1                      # 59: offset of g[0,0] inside padded tile
TAP_OFF = [i * WP + j for i in range(K) for j in range(K)]

# chunks of the [128, SV] grad_in PSUM accumulation (bank-aligned fp32)
PS_CHUNKS = [(0, 1536), (1536, 1536), (3072, SV - 3072)]

# which engine reduces each tap's product (9 taps + 1 bias tap)
# 'd' = VectorE tensor_scalar accum (1x, ~3.5us/pair)
# 'a' = ScalarE activation accum   (~3.7us/pair, parallel engine)
# 'p' = TensorE identity-fold to one PSUM bank + VectorE mini-reduce
#       (~1.4us PE + 0.7us DVE per pair)
# 't' = VectorE 2x fold-tree (tensor_tensor adds) + small accum reduce
# 'apapapapap' (5 ScalarE + 5 TensorE-fold) measured best on HW.
import os
REDUCE_ENG = list(os.environ.get("REDUCE_ENG", "apapapapap"))

_compiled = {}


def _split_sync_waits(bir_bytes, maxw=1):
    """walrus in this env encodes at most one sync-wait per instruction;
    split any multi-wait instruction into preceding single-wait Drains."""
    import orjson
    m = orjson.loads(bir_bytes)
    n_new = [0]

    def fix_block(bb):
        insts = bb.get("instructions")
        if not isinstance(insts, list):
            return
        out = []
        for inst in insts:
            si = inst.get("sync_info")
            waits = (si or {}).get("on_wait") or []
            if len(waits) > maxw:
                excess = waits[:-maxw]
                inst["sync_info"]["on_wait"] = waits[-maxw:]
                for k in range(0, len(excess), maxw):
                    n_new[0] += 1
                    filler = {
                        "name": f"I-wsplit-{n_new[0]}",
                        "opcode": "NoOp",
                        "engine": inst["engine"],
                        "ins": [], "outs": [],
                        "sync_info": {"on_update": [],
                                      "on_wait": excess[k:k + maxw]},
                    }
                    if "debug" in inst:
                        filler["debug"] = inst["debug"]
                    out.append(filler)
            out.append(inst)
        bb["instructions"] = out

    def walk(o):
        if isinstance(o, dict):
            if "instructions" in o:
                fix_block(o)
            for v in o.values():
                walk(v)
        elif isinstance(o, list):
            for v in o:
                walk(v)

    walk(m)
    return orjson.dumps(m), n_new[0]


def _build_kernel():
    import concourse.bass as bass
    import concourse.tile as tile
    import concourse.mybir as mybir

    dt = mybir.dt
    nc = bass.Bass("TRN2", target_bir_lowering=False, debug=False,
                   enable_asserts=False, num_devices=NCORES)

    g_in = nc.dram_tensor("g", [NPAIR, 128, S_TILE], dt.bfloat16,
                          kind="ExternalInput").ap()
    x_in = nc.dram_tensor("x", [NPAIR, 128, S_TILE], dt.bfloat16,
                          kind="ExternalInput").ap()
    w_in = nc.dram_tensor("wdiag", [128, 19 * 128], dt.bfloat16,
                          kind="ExternalInput").ap()
    gin_out = nc.dram_tensor("gin", [NPAIR, 128, SV], dt.float32,
                             kind="ExternalOutput").ap()
    acc_out = nc.dram_tensor("gwacc", [128, NPAIR * 10], dt.float32,
                             kind="ExternalOutput").ap()

    P_TAPS = [t for t in range(10) if REDUCE_ENG[t] == 'p']
    GI_CHUNKS = [(0, 1024), (1024, 1024), (2048, 1024), (3072, SV - 3072)]

    with tile.TileContext(nc) as tc, ExitStack() as ctx:
        wpool = ctx.enter_context(tc.tile_pool(name="w", bufs=1))
        gpool = ctx.enter_context(tc.tile_pool(name="g", bufs=3))
        xpool = ctx.enter_context(tc.tile_pool(name="x", bufs=3))
        ppool = ctx.enter_context(tc.tile_pool(name="prod", bufs=2))
        opool = ctx.enter_context(tc.tile_pool(name="gout", bufs=2))
        apool = ctx.enter_context(tc.tile_pool(name="accp", bufs=1))
        pspool = ctx.enter_context(tc.tile_pool(name="ps", bufs=2,
                                                space="PSUM"))

        wsb = wpool.tile([128, 19 * 128], dt.bfloat16)
        # half-0 diags first so pair 0's matmuls can start immediately
        nc.sync.dma_start(out=wsb[:, 0:9 * 128], in_=w_in[:, 0:9 * 128])
        nc.sync.dma_start(out=wsb[:, 18 * 128:], in_=w_in[:, 18 * 128:])
        nc.sync.dma_start(out=wsb[:, 9 * 128:18 * 128],
                          in_=w_in[:, 9 * 128:18 * 128])
        ident = wsb[:, 18 * 128:19 * 128]

        acc = apool.tile([128, NPAIR * 10], dt.float32)

        Gs, Xs = {}, {}
        prods = {}   # (pair, tap) -> product tile kept for a later stage
        folds = {}   # (pair, tap) -> PSUM fold tile

        def load(pair):
            G = gpool.tile([128, S_TILE], dt.bfloat16, tag="g",
                           name=f"G{pair}")
            nc.sync.dma_start(out=G[:], in_=g_in[pair])
            X = xpool.tile([128, S_TILE], dt.bfloat16, tag="x",
                           name=f"X{pair}")
            nc.sync.dma_start(out=X[:], in_=x_in[pair])
            Gs[pair], Xs[pair] = G, X

        def emit_minis(pair):
            # small PSUM->scalar reduces for last pair's folds (VectorE)
            for t in P_TAPS:
                ps = folds.pop((pair, t))
                slot = acc[:, pair * 10 + t: pair * 10 + t + 1]
                nc.vector.tensor_scalar(ps[:], ps[:], 1.0, None,
                                        op0=mybir.AluOpType.mult,
                                        op1=mybir.AluOpType.add,
                                        accum_out=slot)

        def emit_act_reduces(pair):
            for t in range(10):
                if REDUCE_ENG[t] != 'a':
                    continue
                prod = prods.pop((pair, t))
                slot = acc[:, pair * 10 + t: pair * 10 + t + 1]
                nc.scalar.activation(prod[:], prod[:],
                                     mybir.ActivationFunctionType.Copy,
                                     accum_out=slot)

        def emit_fold_one(pair, t):
            if t < 9:
                src = prods.pop((pair, t))
                src = src[:, 0:SV]
            else:
                src = Gs[pair][:, OFF_C:OFF_C + SV]
            ps = pspool.tile([128, 512], dt.float32, tag="fold",
                             bufs=4, name=f"fold{pair}_{t}")
            nf = (SV + 511) // 512
            for k in range(nf):
                n0 = k * 512
                nn = min(512, SV - n0)
                nc.tensor.matmul(ps[:, 0:nn], lhsT=ident,
                                 rhs=src[:, n0:n0 + nn],
                                 start=(k == 0), stop=(k == nf - 1))
            folds[(pair, t)] = ps

        def emit_mini_one(pair, t):
            ps = folds.pop((pair, t))
            slot = acc[:, pair * 10 + t: pair * 10 + t + 1]
            nc.vector.tensor_scalar(ps[:], ps[:], 1.0, None,
                                    op0=mybir.AluOpType.mult,
                                    op1=mybir.AluOpType.add,
                                    accum_out=slot)

        def emit_ared_one(pair, t):
            prod = prods.pop((pair, t))
            slot = acc[:, pair * 10 + t: pair * 10 + t + 1]
            nc.scalar.activation(prod[:], prod[:],
                                 mybir.ActivationFunctionType.Copy,
                                 accum_out=slot)

        def emit_folds(pair):
            # identity-matmul folds [128,SV] -> one PSUM bank (TensorE)
            for t in P_TAPS:
                emit_fold_one(pair, t)

        def emit_gradin(pair):
            half = pair % 2
            G = Gs[pair]
            gin_sb = opool.tile([128, SV], dt.float32, tag="gin",
                                name=f"gin{pair}")
            for (c0, cn) in GI_CHUNKS:
                ps = pspool.tile([128, cn], dt.float32, tag="ps",
                                 padded_shape=[128, 1024],
                                 name=f"ps{pair}_{c0}")
                for t in range(K * K):
                    lhsT = wsb[:, (half * 9 + t) * 128:
                               (half * 9 + t + 1) * 128]
                    off = TAP_OFF[t] + c0
                    for n0 in range(0, cn, 512):
                        nn = min(512, cn - n0)
                        nc.tensor.matmul(
                            ps[:, n0:n0 + nn], lhsT=lhsT,
                            rhs=G[:, off + n0: off + n0 + nn],
                            start=(t == 0), stop=(t == K * K - 1))
                nc.scalar.copy(gin_sb[:, c0:c0 + cn], ps[:])
            nc.sync.dma_start(out=gin_out[pair], in_=gin_sb[:])

        def emit_mults(pair):
            G, X = Gs[pair], Xs[pair]
            for t in range(K * K):
                kind = REDUCE_ENG[t]
                prod = ppool.tile([128, SV], dt.bfloat16,
                                  tag=f"prod_{kind}",
                                  bufs=(10 if kind == 'p' else
                                        8 if kind == 'a' else 2),
                                  name=f"prod{pair}_{t}")
                nc.vector.tensor_mul(prod[:],
                                     X[:, TAP_OFF[t]:TAP_OFF[t] + SV],
                                     G[:, OFF_C:OFF_C + SV])
                slot = acc[:, pair * 10 + t: pair * 10 + t + 1]
                if kind == 'd':
                    nc.vector.tensor_scalar(prod[:], prod[:], 1.0, None,
                                            op0=mybir.AluOpType.mult,
                                            op1=mybir.AluOpType.add,
                                            accum_out=slot)
                elif kind == 't':
                    nc.vector.tensor_add(prod[:, 0:1624], prod[:, 0:1624],
                                         prod[:, 1624:3248])
                    nc.vector.tensor_add(prod[:, 0:812], prod[:, 0:812],
                                         prod[:, 812:1624])
                    nc.vector.tensor_add(prod[:, 0:406], prod[:, 0:406],
                                         prod[:, 406:812])
                    nc.vector.tensor_scalar(prod[:, 0:406], prod[:, 0:406],
                                            1.0, None,
                                            op0=mybir.AluOpType.mult,
                                            op1=mybir.AluOpType.add,
                                            accum_out=slot)
                else:
                    prods[(pair, t)] = prod
            bkind = REDUCE_ENG[9]
            slot = acc[:, pair * 10 + 9: pair * 10 + 10]
            if bkind == 'd':
                bsc = ppool.tile([128, SV], dt.bfloat16, tag="prod_d",
                                 bufs=2, name=f"bias{pair}")
                nc.vector.tensor_scalar(bsc[:], G[:, OFF_C:OFF_C + SV], 1.0,
                                        None, op0=mybir.AluOpType.mult,
                                        op1=mybir.AluOpType.add,
                                        accum_out=slot)
            elif bkind == 't':
                bsc = ppool.tile([128, 1624], dt.bfloat16, tag="prod_t2",
                                 bufs=2, name=f"bias{pair}")
                nc.vector.tensor_add(bsc[:], G[:, OFF_C:OFF_C + 1624],
                                     G[:, OFF_C + 1624:OFF_C + 3248])
                nc.vector.tensor_add(bsc[:, 0:812], bsc[:, 0:812],
                                     bsc[:, 812:1624])
                nc.vector.tensor_add(bsc[:, 0:406], bsc[:, 0:406],
                                     bsc[:, 406:812])
                nc.vector.tensor_scalar(bsc[:, 0:406], bsc[:, 0:406],
                                        1.0, None,
                                        op0=mybir.AluOpType.mult,
                                        op1=mybir.AluOpType.add,
                                        accum_out=slot)
            elif bkind == 'a':
                bsc = ppool.tile([128, SV], dt.bfloat16, tag="prod_a",
                                 bufs=8, name=f"bias{pair}")
                nc.vector.tensor_copy(bsc[:], G[:, OFF_C:OFF_C + SV])
                prods[(pair, 9)] = bsc

        # software pipeline: pair p's folds/minis/ACT-reduces are issued at
        # the head of pair p+1's block so no engine waits on same-block deps
        load(0)
        load(1)
        for pair in range(NPAIR):
            if pair + 2 < NPAIR:
                load(pair + 2)
            if pair > 0:
                emit_folds(pair - 1)
                emit_minis(pair - 1)
                emit_act_reduces(pair - 1)
            emit_gradin(pair)
            emit_mults(pair)
        # tail: interleave the last pair's reduces per tap so the
        # pipeline drains with engines chasing each other
        last = NPAIR - 1
        for t in range(10):
            if (last, t) in prods or (REDUCE_ENG[t] == 'p' and t == 9):
                if REDUCE_ENG[t] == 'p':
                    emit_fold_one(last, t)
                    emit_mini_one(last, t)
                elif REDUCE_ENG[t] == 'a':
                    emit_ared_one(last, t)

        nc.sync.dma_start(out=acc_out[:], in_=acc[:])

    patched, n_fill = _split_sync_waits(nc.to_json_bytes())
    nc.to_json_bytes = lambda: patched
    return nc


def _host_prep(g_out, inputs, kernels):
    """Transpose to [img, c, padded-spatial] bf16 + build diag weights."""
    bf16 = ml_dtypes.bfloat16

    def to_tiles(a):
        # [B,H,W,C] fp32 -> [B, C, HP*WP(+pad)] bf16 with zero borders
        t = np.zeros((B, C, HP, WP), dtype=bf16)
        t[:, :, 1:H + 1, 1:W + 1] = a.transpose(0, 3, 1, 2)
        t = t.reshape(B, C, S_PAD)
        out = np.zeros((B, C, S_TILE), dtype=bf16)
        out[:, :, :S_PAD] = t
        # -> per-core [NPAIR, 128, S_TILE]; pair = img_local*2 + half
        out = out.reshape(B, 2, 128, S_TILE)
        return [out[c * IMG_PER_CORE:(c + 1) * IMG_PER_CORE].reshape(
            NPAIR, 128, S_TILE) for c in range(NCORES)]

    g_tiles = to_tiles(g_out)
    x_tiles = to_tiles(inputs)

    # rotated weights for grad_in: wr[i,j,c] = kernels[2-i,2-j,c,0]
    wr = kernels[::-1, ::-1, :, 0]          # [3,3,C]
    wsb = np.zeros((128, 19 * 128), dtype=bf16)
    for t in range(K * K):
        i, j = t // K, t % K
        for hhalf in range(2):
            blk = (hhalf * 9 + t) * 128
            wsb[np.arange(128), blk + np.arange(128)] = \
                wr[i, j, hhalf * 128:(hhalf + 1) * 128].astype(bf16)
    # block 18: identity (for TensorE fold-reductions)
    wsb[np.arange(128), 18 * 128 + np.arange(128)] = bf16(1.0)
    return g_tiles, x_tiles, wsb


def _run(in_maps, trace=False, tmpdir=None):
    from concourse.bass_utils import run_bass_kernel_spmd
    if "nc" not in _compiled:
        _compiled["nc"] = _build_kernel()
    nc = _compiled["nc"]
    return run_bass_kernel_spmd(nc, in_maps, list(range(NCORES)),
                                trace=trace, tmpdir=tmpdir)


def kernel(g_out, inputs, kernels, _trace=False, _tmpdir=None):
    g_out = np.asarray(g_out, dtype=np.float32)
    inputs = np.asarray(inputs, dtype=np.float32)
    kernels = np.asarray(kernels, dtype=np.float32)

    g_tiles, x_tiles, wsb = _host_prep(g_out, inputs, kernels)
    in_maps = [{"g": g_tiles[c], "x": x_tiles[c], "wdiag": wsb}
               for c in range(NCORES)]
    res = _run(in_maps, trace=_trace, tmpdir=_tmpdir)
    results = res.results
    kernel._last_exec_time_ns = res.exec_time_ns

    # ---- unshard / assemble ----
    grad_in = np.empty((B, C, H, W), dtype=np.float32)
    gw_acc = np.zeros((128, NPAIR * 10), dtype=np.float64)
    for c in range(NCORES):
        gin = results[c]["gin"]          # [NPAIR, 128, SV] f32
        gin = gin.reshape(NPAIR, 128, H, WP)[:, :, :, :W]
        for p in range(NPAIR):
            img = c * IMG_PER_CORE + p // 2
            hhalf = p % 2
            grad_in[img, hhalf * 128:(hhalf + 1) * 128] = gin[p]
        gw_acc += results[c]["gwacc"].astype(np.float64)
    grad_in = grad_in.transpose(0, 2, 3, 1)

    # gw_acc[c_part, pair*10 + t]: sum over images (pairs with same half)
    gw_acc = gw_acc.reshape(128, IMG_PER_CORE, 2, 10).sum(axis=1)
    grad_wt = np.empty((K, K, C, 1), dtype=np.float32)
    grad_bias = np.empty((C,), dtype=np.float32)
    for hhalf in range(2):
        sl = slice(hhalf * 128, (hhalf + 1) * 128)
        for t in range(K * K):
            grad_wt[t // K, t % K, sl, 0] = gw_acc[:, hhalf, t]
        grad_bias[sl] = gw_acc[:, hhalf, 9]

    return (grad_in, grad_wt.astype(np.float32), grad_bias.astype(np.float32))


# revision 24
# speedup vs baseline: 1.0071x; 1.0071x over previous
"""Trainium2 Bass kernel: BackwardInjectDepthwiseConv2D (B=32,H=W=56,C=256,K=3).

Outputs (grad_in, grad_wt, grad_bias) for a depthwise conv backward:
  grad_in[b,y,x,c]  = sum_{i,j} g_pad[b,y+i,x+j,c] * kernels[2-i,2-j,c,0]
  grad_wt[i,j,c]    = sum_{b,y,x} x_pad[b,y+i,x+j,c] * g_out[b,y,x,c]
  grad_bias[c]      = sum_{b,y,x} g_out[b,y,x,c]

Strategy (8 NeuronCores, pure batch data-parallel, 4 images/core):
 - Host reshapes to channels-on-partitions layout [img, c, 58*58] (zero
   padded borders) in bf16; per core 8 "pairs" = 4 images x 2 channel
   halves of 128.
 - grad_in on TensorE: 9 accumulating diag-matmuls per pair (lhsT =
   diag(w_tap) per channel-half); shifted taps are free-dim slices of
   the padded tile.  PSUM fp32 accumulation, ScalarE evacuates.
 - grad_wt: per tap a bf16 tensor_tensor multiply on VectorE (2x mode);
   the per-partition free-dim reduction of each product is split across
   ScalarE (activation accum_out, 1x) and TensorE (identity-matmul fold
   into one PSUM bank + small VectorE accum) to balance all three
   engines (~85% busy each).
 - grad_bias rides the same reduce path on g directly.
 - Weight/bias grads are per-core partials summed on host; the three
   pipeline stages per pair are software-pipelined one pair apart so
   no engine waits on same-block producers.
 - BIR post-pass splits multi-sem-wait instructions into single-wait
   NoOp prefixes (this walrus build encodes at most one wait/inst).
"""

import sys

for _p in ("/opt/trn_rl_repo",):
    if _p not in sys.path:
        sys.path.insert(0, _p)

import numpy as np
import ml_dtypes
from contextlib import ExitStack

B, H, W, C, K = 32, 56, 56, 256, 3
NCORES = 8
IMG_PER_CORE = B // NCORES          # 4
NPAIR = IMG_PER_CORE * 2            # 4 images x 2 channel halves
HP = H + 2                          # 58 (padded rows)
WP = W + 2                          # 58 (padded cols)
S_PAD = HP * WP                     # 3364
S_TILE = 3368                       # tile free size (tail pad, even)
SV = H * WP                         # 3248: output window (56 rows of 58)
OFF_C = WP + 1                      # 59: offset of g[0,0] inside padded tile
TAP_OFF = [i * WP + j for i in range(K) for j in range(K)]

# chunks of the [128, SV] grad_in PSUM accumulation (bank-aligned fp32)
PS_CHUNKS = [(0, 1536), (1536, 1536), (3072, SV - 3072)]

# which engine reduces each tap's product (9 taps + 1 bias tap)
# 'd' = VectorE tensor_scalar accum (1x, ~3.5us/pair)
# 'a' = ScalarE activation accum   (~3.7us/pair, parallel engine)
# 'p' = TensorE identity-fold to one PSUM bank + VectorE mini-reduce
#       (~1.4us PE + 0.7us DVE per pair)
# 't' = VectorE 2x fold-tree (tensor_tensor adds) + small accum reduce
# 'apapapapap' (5 ScalarE + 5 TensorE-fold) measured best on HW.
import os
REDUCE_ENG = list(os.environ.get("REDUCE_ENG", "apapapapap"))

_compiled = {}


def _split_sync_waits(bir_bytes, maxw=1):
    """walrus in this env encodes at most one sync-wait per instruction;
    split any multi-wait instruction into preceding single-wait Drains."""
    import orjson
    m = orjson.loads(bir_bytes)
    n_new = [0]

    def fix_block(bb):
        insts = bb.get("instructions")
        if not isinstance(insts, list):
            return
        out = []
        for inst in insts:
            si = inst.get("sync_info")
            waits = (si or {}).get("on_wait") or []
            if len(waits) > maxw:
                excess = waits[:-maxw]
                inst["sync_info"]["on_wait"] = waits[-maxw:]
                for k in range(0, len(excess), maxw):
                    n_new[0] += 1
                    filler = {
                        "name": f"I-wsplit-{n_new[0]}",
                        "opcode": "NoOp",
                        "engine": inst["engine"],
                        "ins": [], "outs": [],
                        "sync_info": {"on_update": [],
                                      "on_wait": excess[k:k + maxw]},
                    }
                    if "debug" in inst:
                        filler["debug"] = inst["debug"]
                    out.append(filler)
            out.append(inst)
        bb["instructions"] = out

    def walk(o):
        if isinstance(o, dict):
            if "instructions" in o:
                fix_block(o)
            for v in o.values():
                walk(v)
        elif isinstance(o, list):
            for v in o:
                walk(v)

    walk(m)
    return orjson.dumps(m), n_new[0]


def _build_kernel():
    import concourse.bass as bass
    import concourse.tile as tile
    import concourse.mybir as mybir

    dt = mybir.dt
    nc = bass.Bass("TRN2", target_bir_lowering=False, debug=False,
                   enable_asserts=False, num_devices=NCORES)

    g_in = nc.dram_tensor("g", [NPAIR, 128, S_TILE], dt.bfloat16,
                          kind="ExternalInput").ap()
    x_in = nc.dram_tensor("x", [NPAIR, 128, S_TILE], dt.bfloat16,
                          kind="ExternalInput").ap()
    w_in = nc.dram_tensor("wdiag", [128, 19 * 128], dt.bfloat16,
                          kind="ExternalInput").ap()
    gin_out = nc.dram_tensor("gin", [NPAIR, 128, SV], dt.float32,
                             kind="ExternalOutput").ap()
    acc_out = nc.dram_tensor("gwacc", [128, NPAIR * 10], dt.float32,
                             kind="ExternalOutput").ap()

    P_TAPS = [t for t in range(10) if REDUCE_ENG[t] == 'p']
    GI_CHUNKS = [(0, 1024), (1024, 1024), (2048, 1024), (3072, SV - 3072)]

    with tile.TileContext(nc) as tc, ExitStack() as ctx:
        wpool = ctx.enter_context(tc.tile_pool(name="w", bufs=1))
        gpool = ctx.enter_context(tc.tile_pool(name="g", bufs=3))
        xpool = ctx.enter_context(tc.tile_pool(name="x", bufs=3))
        ppool = ctx.enter_context(tc.tile_pool(name="prod", bufs=2))
        opool = ctx.enter_context(tc.tile_pool(name="gout", bufs=2))
        apool = ctx.enter_context(tc.tile_pool(name="accp", bufs=1))
        pspool = ctx.enter_context(tc.tile_pool(name="ps", bufs=2,
                                                space="PSUM"))

        wsb = wpool.tile([128, 19 * 128], dt.bfloat16)
        # half-0 diags first so pair 0's matmuls can start immediately;
        # half-1 diags are loaded after the first data tiles (see below)
        nc.sync.dma_start(out=wsb[:, 0:9 * 128], in_=w_in[:, 0:9 * 128])
        nc.sync.dma_start(out=wsb[:, 18 * 128:], in_=w_in[:, 18 * 128:])
        ident = wsb[:, 18 * 128:19 * 128]

        acc = apool.tile([128, NPAIR * 10], dt.float32)

        Gs, Xs = {}, {}
        prods = {}   # (pair, tap) -> product tile kept for a later stage
        folds = {}   # (pair, tap) -> PSUM fold tile

        def load(pair):
            G = gpool.tile([128, S_TILE], dt.bfloat16, tag="g",
                           name=f"G{pair}")
            nc.sync.dma_start(out=G[:], in_=g_in[pair])
            X = xpool.tile([128, S_TILE], dt.bfloat16, tag="x",
                           name=f"X{pair}")
            nc.sync.dma_start(out=X[:], in_=x_in[pair])
            Gs[pair], Xs[pair] = G, X

        def emit_minis(pair):
            # small PSUM->scalar reduces for last pair's folds (VectorE)
            for t in P_TAPS:
                ps = folds.pop((pair, t))
                slot = acc[:, pair * 10 + t: pair * 10 + t + 1]
                nc.vector.tensor_scalar(ps[:], ps[:], 1.0, None,
                                        op0=mybir.AluOpType.mult,
                                        op1=mybir.AluOpType.add,
                                        accum_out=slot)

        def emit_act_reduces(pair):
            for t in range(10):
                if REDUCE_ENG[t] != 'a':
                    continue
                prod = prods.pop((pair, t))
                slot = acc[:, pair * 10 + t: pair * 10 + t + 1]
                nc.scalar.activation(prod[:], prod[:],
                                     mybir.ActivationFunctionType.Copy,
                                     accum_out=slot)

        def emit_fold_one(pair, t):
            if t < 9:
                src = prods.pop((pair, t))
                src = src[:, 0:SV]
            else:
                src = Gs[pair][:, OFF_C:OFF_C + SV]
            ps = pspool.tile([128, 512], dt.float32, tag="fold",
                             bufs=4, name=f"fold{pair}_{t}")
            nf = (SV + 511) // 512
            for k in range(nf):
                n0 = k * 512
                nn = min(512, SV - n0)
                nc.tensor.matmul(ps[:, 0:nn], lhsT=ident,
                                 rhs=src[:, n0:n0 + nn],
                                 start=(k == 0), stop=(k == nf - 1))
            folds[(pair, t)] = ps

        def emit_mini_one(pair, t):
            ps = folds.pop((pair, t))
            slot = acc[:, pair * 10 + t: pair * 10 + t + 1]
            nc.vector.tensor_scalar(ps[:], ps[:], 1.0, None,
                                    op0=mybir.AluOpType.mult,
                                    op1=mybir.AluOpType.add,
                                    accum_out=slot)

        def emit_ared_one(pair, t):
            prod = prods.pop((pair, t))
            slot = acc[:, pair * 10 + t: pair * 10 + t + 1]
            nc.scalar.activation(prod[:], prod[:],
                                 mybir.ActivationFunctionType.Copy,
                                 accum_out=slot)

        def emit_folds(pair):
            # identity-matmul folds [128,SV] -> one PSUM bank (TensorE)
            for t in P_TAPS:
                emit_fold_one(pair, t)

        def emit_gradin(pair):
            half = pair % 2
            G = Gs[pair]
            gin_sb = opool.tile([128, SV], dt.float32, tag="gin",
                                name=f"gin{pair}")
            for (c0, cn) in GI_CHUNKS:
                ps = pspool.tile([128, cn], dt.float32, tag="ps",
                                 padded_shape=[128, 1024],
                                 name=f"ps{pair}_{c0}")
                for t in range(K * K):
                    lhsT = wsb[:, (half * 9 + t) * 128:
                               (half * 9 + t + 1) * 128]
                    off = TAP_OFF[t] + c0
                    for n0 in range(0, cn, 512):
                        nn = min(512, cn - n0)
                        nc.tensor.matmul(
                            ps[:, n0:n0 + nn], lhsT=lhsT,
                            rhs=G[:, off + n0: off + n0 + nn],
                            start=(t == 0), stop=(t == K * K - 1))
                nc.scalar.copy(gin_sb[:, c0:c0 + cn], ps[:])
            nc.sync.dma_start(out=gin_out[pair], in_=gin_sb[:])

        def emit_mults(pair):
            G, X = Gs[pair], Xs[pair]
            for t in range(K * K):
                kind = REDUCE_ENG[t]
                prod = ppool.tile([128, SV], dt.bfloat16,
                                  tag=f"prod_{kind}",
                                  bufs=(10 if kind == 'p' else
                                        8 if kind == 'a' else 2),
                                  name=f"prod{pair}_{t}")
                nc.vector.tensor_mul(prod[:],
                                     X[:, TAP_OFF[t]:TAP_OFF[t] + SV],
                                     G[:, OFF_C:OFF_C + SV])
                slot = acc[:, pair * 10 + t: pair * 10 + t + 1]
                if kind == 'd':
                    nc.vector.tensor_scalar(prod[:], prod[:], 1.0, None,
                                            op0=mybir.AluOpType.mult,
                                            op1=mybir.AluOpType.add,
                                            accum_out=slot)
                elif kind == 't':
                    nc.vector.tensor_add(prod[:, 0:1624], prod[:, 0:1624],
                                         prod[:, 1624:3248])
                    nc.vector.tensor_add(prod[:, 0:812], prod[:, 0:812],
                                         prod[:, 812:1624])
                    nc.vector.tensor_add(prod[:, 0:406], prod[:, 0:406],
                                         prod[:, 406:812])
                    nc.vector.tensor_scalar(prod[:, 0:406], prod[:, 0:406],
                                            1.0, None,
                                            op0=mybir.AluOpType.mult,
                                            op1=mybir.AluOpType.add,
                                            accum_out=slot)
                else:
                    prods[(pair, t)] = prod
            bkind = REDUCE_ENG[9]
            slot = acc[:, pair * 10 + 9: pair * 10 + 10]
            if bkind == 'd':
                bsc = ppool.tile([128, SV], dt.bfloat16, tag="prod_d",
                                 bufs=2, name=f"bias{pair}")
                nc.vector.tensor_scalar(bsc[:], G[:, OFF_C:OFF_C + SV], 1.0,
                                        None, op0=mybir.AluOpType.mult,
                                        op1=mybir.AluOpType.add,
                                        accum_out=slot)
            elif bkind == 't':
                bsc = ppool.tile([128, 1624], dt.bfloat16, tag="prod_t2",
                                 bufs=2, name=f"bias{pair}")
                nc.vector.tensor_add(bsc[:], G[:, OFF_C:OFF_C + 1624],
                                     G[:, OFF_C + 1624:OFF_C + 3248])
                nc.vector.tensor_add(bsc[:, 0:812], bsc[:, 0:812],
                                     bsc[:, 812:1624])
                nc.vector.tensor_add(bsc[:, 0:406], bsc[:, 0:406],
                                     bsc[:, 406:812])
                nc.vector.tensor_scalar(bsc[:, 0:406], bsc[:, 0:406],
                                        1.0, None,
                                        op0=mybir.AluOpType.mult,
                                        op1=mybir.AluOpType.add,
                                        accum_out=slot)
            elif bkind == 'a':
                bsc = ppool.tile([128, SV], dt.bfloat16, tag="prod_a",
                                 bufs=8, name=f"bias{pair}")
                nc.vector.tensor_copy(bsc[:], G[:, OFF_C:OFF_C + SV])
                prods[(pair, 9)] = bsc

        # software pipeline: pair p's folds/minis/ACT-reduces are issued at
        # the head of pair p+1's block so no engine waits on same-block deps
        load(0)
        nc.sync.dma_start(out=wsb[:, 9 * 128:18 * 128],
                          in_=w_in[:, 9 * 128:18 * 128])
        load(1)
        for pair in range(NPAIR):
            if pair + 2 < NPAIR:
                load(pair + 2)
            if pair == NPAIR - 1:
                # flush completed pairs' accumulators under the last block
                nc.sync.dma_start(out=acc_out[:, 0:60], in_=acc[:, 0:60])
            if pair > 0:
                emit_folds(pair - 1)
                emit_minis(pair - 1)
                emit_act_reduces(pair - 1)
            emit_gradin(pair)
            emit_mults(pair)
        # tail: interleave the last pair's reduces per tap so the
        # pipeline drains with engines chasing each other
        last = NPAIR - 1
        for t in range(10):
            if (last, t) in prods or (REDUCE_ENG[t] == 'p' and t == 9):
                if REDUCE_ENG[t] == 'p':
                    emit_fold_one(last, t)
                    emit_mini_one(last, t)
                elif REDUCE_ENG[t] == 'a':
                    emit_ared_one(last, t)

        nc.sync.dma_start(out=acc_out[:, 60:], in_=acc[:, 60:])

    patched, n_fill = _split_sync_waits(nc.to_json_bytes())
    nc.to_json_bytes = lambda: patched
    return nc


def _host_prep(g_out, inputs, kernels):
    """Transpose to [img, c, padded-spatial] bf16 + build diag weights."""
    bf16 = ml_dtypes.bfloat16

    def to_tiles(a):
        # [B,H,W,C] fp32 -> [B, C, HP*WP(+pad)] bf16 with zero borders
        t = np.zeros((B, C, HP, WP), dtype=bf16)
        t[:, :, 1:H + 1, 1:W + 1] = a.transpose(0, 3, 1, 2)
        t = t.reshape(B, C, S_PAD)
        out = np.zeros((B, C, S_TILE), dtype=bf16)
        out[:, :, :S_PAD] = t
        # -> per-core [NPAIR, 128, S_TILE]; pair = img_local*2 + half
        out = out.reshape(B, 2, 128, S_TILE)
        return [out[c * IMG_PER_CORE:(c + 1) * IMG_PER_CORE].reshape(
            NPAIR, 128, S_TILE) for c in range(NCORES)]

    g_tiles = to_tiles(g_out)
    x_tiles = to_tiles(inputs)

    # rotated weights for grad_in: wr[i,j,c] = kernels[2-i,2-j,c,0]
    wr = kernels[::-1, ::-1, :, 0]          # [3,3,C]
    wsb = np.zeros((128, 19 * 128), dtype=bf16)
    for t in range(K * K):
        i, j = t // K, t % K
        for hhalf in range(2):
            blk = (hhalf * 9 + t) * 128
            wsb[np.arange(128), blk + np.arange(128)] = \
                wr[i, j, hhalf * 128:(hhalf + 1) * 128].astype(bf16)
    # block 18: identity (for TensorE fold-reductions)
    wsb[np.arange(128), 18 * 128 + np.arange(128)] = bf16(1.0)
    return g_tiles, x_tiles, wsb


def _run(in_maps, trace=False, tmpdir=None):
    from concourse.bass_utils import run_bass_kernel_spmd
    if "nc" not in _compiled:
        _compiled["nc"] = _build_kernel()
    nc = _compiled["nc"]
    return run_bass_kernel_spmd(nc, in_maps, list(range(NCORES)),
                                trace=trace, tmpdir=tmpdir)


def kernel(g_out, inputs, kernels, _trace=False, _tmpdir=None):
    g_out = np.asarray(g_out, dtype=np.float32)
    inputs = np.asarray(inputs, dtype=np.float32)
    kernels = np.asarray(kernels, dtype=np.float32)

    g_tiles, x_tiles, wsb = _host_prep(g_out, inputs, kernels)
    in_maps = [{"g": g_tiles[c], "x": x_tiles[c], "wdiag": wsb}
               for c in range(NCORES)]
    res = _run(in_maps, trace=_trace, tmpdir=_tmpdir)
    results = res.results
    kernel._last_exec_time_ns = res.exec_time_ns

    # ---- unshard / assemble ----
    grad_in = np.empty((B, C, H, W), dtype=np.float32)
    gw_acc = np.zeros((128, NPAIR * 10), dtype=np.float64)
    for c in range(NCORES):
        gin = results[c]["gin"]          # [NPAIR, 128, SV] f32
        gin = gin.reshape(NPAIR, 128, H, WP)[:, :, :, :W]
        for p in range(NPAIR):
            img = c * IMG_PER_CORE + p // 2
            hhalf = p % 2
            grad_in[img, hhalf * 128:(hhalf + 1) * 128] = gin[p]
        gw_acc += results[c]["gwacc"].astype(np.float64)
    grad_in = grad_in.transpose(0, 2, 3, 1)

    # gw_acc[c_part, pair*10 + t]: sum over images (pairs with same half)
    gw_acc = gw_acc.reshape(128, IMG_PER_CORE, 2, 10).sum(axis=1)
    grad_wt = np.empty((K, K, C, 1), dtype=np.float32)
    grad_bias = np.empty((C,), dtype=np.float32)
    for hhalf in range(2):
        sl = slice(hhalf * 128, (hhalf + 1) * 128)
        for t in range(K * K):
            grad_wt[t // K, t % K, sl, 0] = gw_acc[:, hhalf, t]
        grad_bias[sl] = gw_acc[:, hhalf, 9]

    return (grad_in, grad_wt.astype(np.float32), grad_bias.astype(np.float32))


# revision 27
# speedup vs baseline: 1.0088x; 1.0017x over previous
"""Trainium2 Bass kernel: BackwardInjectDepthwiseConv2D (B=32,H=W=56,C=256,K=3).

Outputs (grad_in, grad_wt, grad_bias) for a depthwise conv backward:
  grad_in[b,y,x,c]  = sum_{i,j} g_pad[b,y+i,x+j,c] * kernels[2-i,2-j,c,0]
  grad_wt[i,j,c]    = sum_{b,y,x} x_pad[b,y+i,x+j,c] * g_out[b,y,x,c]
  grad_bias[c]      = sum_{b,y,x} g_out[b,y,x,c]

Strategy (8 NeuronCores, pure batch data-parallel, 4 images/core):
 - Host reshapes to channels-on-partitions layout [img, c, 58*58] (zero
   padded borders) in bf16; per core 8 "pairs" = 4 images x 2 channel
   halves of 128.
 - grad_in on TensorE: 9 accumulating diag-matmuls per pair (lhsT =
   diag(w_tap) per channel-half); shifted taps are free-dim slices of
   the padded tile.  PSUM fp32 accumulation, ScalarE evacuates.
 - grad_wt: per tap a bf16 tensor_tensor multiply on VectorE (2x mode);
   the per-partition free-dim reduction of each product is split across
   ScalarE (activation accum_out, 1x) and TensorE (identity-matmul fold
   into one PSUM bank + small VectorE accum) to balance all three
   engines (~85% busy each).
 - grad_bias rides the same reduce path on g directly.
 - Weight/bias grads are per-core partials summed on host; the three
   pipeline stages per pair are software-pipelined one pair apart so
   no engine waits on same-block producers.
 - BIR post-pass splits multi-sem-wait instructions into single-wait
   NoOp prefixes (this walrus build encodes at most one wait/inst).
"""

import sys

for _p in ("/opt/trn_rl_repo",):
    if _p not in sys.path:
        sys.path.insert(0, _p)

import numpy as np
import ml_dtypes
from contextlib import ExitStack

B, H, W, C, K = 32, 56, 56, 256, 3
NCORES = 8
IMG_PER_CORE = B // NCORES          # 4
NPAIR = IMG_PER_CORE * 2            # 4 images x 2 channel halves
HP = H + 2                          # 58 (padded rows)
WP = W + 2                          # 58 (padded cols)
S_PAD = HP * WP                     # 3364
S_TILE = 3368                       # tile free size (tail pad, even)
SV = H * WP                         # 3248: output window (56 rows of 58)
OFF_C = WP + 1                      # 59: offset of g[0,0] inside padded tile
TAP_OFF = [i * WP + j for i in range(K) for j in range(K)]

# chunks of the [128, SV] grad_in PSUM accumulation (bank-aligned fp32)
PS_CHUNKS = [(0, 1536), (1536, 1536), (3072, SV - 3072)]

# which engine reduces each tap's product (9 taps + 1 bias tap)
# 'd' = VectorE tensor_scalar accum (1x, ~3.5us/pair)
# 'a' = ScalarE activation accum   (~3.7us/pair, parallel engine)
# 'p' = TensorE identity-fold to one PSUM bank + VectorE mini-reduce
#       (~1.4us PE + 0.7us DVE per pair)
# 't' = VectorE 2x fold-tree (tensor_tensor adds) + small accum reduce
# 'apapapapap' (5 ScalarE + 5 TensorE-fold) measured best on HW.
import os
REDUCE_ENG = list(os.environ.get("REDUCE_ENG", "apapapapap"))

_compiled = {}


def _split_sync_waits(bir_bytes, maxw=1):
    """walrus in this env encodes at most one sync-wait per instruction;
    split any multi-wait instruction into preceding single-wait Drains."""
    import orjson
    m = orjson.loads(bir_bytes)
    n_new = [0]

    def fix_block(bb):
        insts = bb.get("instructions")
        if not isinstance(insts, list):
            return
        out = []
        for inst in insts:
            si = inst.get("sync_info")
            waits = (si or {}).get("on_wait") or []
            if len(waits) > maxw:
                excess = waits[:-maxw]
                inst["sync_info"]["on_wait"] = waits[-maxw:]
                for k in range(0, len(excess), maxw):
                    n_new[0] += 1
                    filler = {
                        "name": f"I-wsplit-{n_new[0]}",
                        "opcode": "NoOp",
                        "engine": inst["engine"],
                        "ins": [], "outs": [],
                        "sync_info": {"on_update": [],
                                      "on_wait": excess[k:k + maxw]},
                    }
                    if "debug" in inst:
                        filler["debug"] = inst["debug"]
                    out.append(filler)
            out.append(inst)
        bb["instructions"] = out

    def walk(o):
        if isinstance(o, dict):
            if "instructions" in o:
                fix_block(o)
            for v in o.values():
                walk(v)
        elif isinstance(o, list):
            for v in o:
                walk(v)

    walk(m)
    return orjson.dumps(m), n_new[0]


def _build_kernel():
    import concourse.bass as bass
    import concourse.tile as tile
    import concourse.mybir as mybir

    dt = mybir.dt
    nc = bass.Bass("TRN2", target_bir_lowering=False, debug=False,
                   enable_asserts=False, num_devices=NCORES)

    g_in = nc.dram_tensor("g", [NPAIR, 128, S_TILE], dt.bfloat16,
                          kind="ExternalInput").ap()
    x_in = nc.dram_tensor("x", [NPAIR, 128, S_TILE], dt.bfloat16,
                          kind="ExternalInput").ap()
    w_in = nc.dram_tensor("wdiag", [128, 19 * 128], dt.bfloat16,
                          kind="ExternalInput").ap()
    gin_out = nc.dram_tensor("gin", [NPAIR, 128, SV], dt.float32,
                             kind="ExternalOutput").ap()
    acc_out = nc.dram_tensor("gwacc", [128, NPAIR * 10], dt.float32,
                             kind="ExternalOutput").ap()

    P_TAPS = [t for t in range(10) if REDUCE_ENG[t] == 'p']
    GI_CHUNKS = [(0, 1024), (1024, 1024), (2048, 1024), (3072, SV - 3072)]

    with tile.TileContext(nc) as tc, ExitStack() as ctx:
        wpool = ctx.enter_context(tc.tile_pool(name="w", bufs=1))
        gpool = ctx.enter_context(tc.tile_pool(name="g", bufs=3))
        xpool = ctx.enter_context(tc.tile_pool(name="x", bufs=3))
        ppool = ctx.enter_context(tc.tile_pool(name="prod", bufs=2))
        opool = ctx.enter_context(tc.tile_pool(name="gout", bufs=2))
        apool = ctx.enter_context(tc.tile_pool(name="accp", bufs=1))
        pspool = ctx.enter_context(tc.tile_pool(name="ps", bufs=2,
                                                space="PSUM"))

        wsb = wpool.tile([128, 19 * 128], dt.bfloat16)
        # tap-0/half-0 weights first so pair 0's first matmuls can start
        # as soon as the head of G0 lands; half-1 diags load after the
        # first data tiles (see below)
        nc.sync.dma_start(out=wsb[:, 0:128], in_=w_in[:, 0:128])
        ident = wsb[:, 18 * 128:19 * 128]

        acc = apool.tile([128, NPAIR * 10], dt.float32)

        Gs, Xs = {}, {}
        prods = {}   # (pair, tap) -> product tile kept for a later stage
        folds = {}   # (pair, tap) -> PSUM fold tile

        def load(pair, split=False):
            G = gpool.tile([128, S_TILE], dt.bfloat16, tag="g",
                           name=f"G{pair}")
            if split:
                # head of G first: unblocks pair 0's chunk-0 matmuls
                nc.sync.dma_start(out=G[:, 0:1792], in_=g_in[pair][:, 0:1792])
                nc.sync.dma_start(out=wsb[:, 128:9 * 128],
                                  in_=w_in[:, 128:9 * 128])
                nc.sync.dma_start(out=G[:, 1792:], in_=g_in[pair][:, 1792:])
                nc.sync.dma_start(out=wsb[:, 18 * 128:],
                                  in_=w_in[:, 18 * 128:])
            else:
                nc.sync.dma_start(out=G[:], in_=g_in[pair])
            X = xpool.tile([128, S_TILE], dt.bfloat16, tag="x",
                           name=f"X{pair}")
            nc.sync.dma_start(out=X[:], in_=x_in[pair])
            Gs[pair], Xs[pair] = G, X

        def emit_minis(pair):
            # small PSUM->scalar reduces for last pair's folds (VectorE)
            for t in P_TAPS:
                ps = folds.pop((pair, t))
                slot = acc[:, pair * 10 + t: pair * 10 + t + 1]
                nc.vector.tensor_scalar(ps[:], ps[:], 1.0, None,
                                        op0=mybir.AluOpType.mult,
                                        op1=mybir.AluOpType.add,
                                        accum_out=slot)

        def emit_act_reduces(pair):
            for t in range(10):
                if REDUCE_ENG[t] != 'a':
                    continue
                prod = prods.pop((pair, t))
                slot = acc[:, pair * 10 + t: pair * 10 + t + 1]
                nc.scalar.activation(prod[:], prod[:],
                                     mybir.ActivationFunctionType.Copy,
                                     accum_out=slot)

        def emit_fold_one(pair, t):
            if t < 9:
                src = prods.pop((pair, t))
                src = src[:, 0:SV]
            else:
                src = Gs[pair][:, OFF_C:OFF_C + SV]
            ps = pspool.tile([128, 512], dt.float32, tag="fold",
                             bufs=4, name=f"fold{pair}_{t}")
            nf = (SV + 511) // 512
            for k in range(nf):
                n0 = k * 512
                nn = min(512, SV - n0)
                nc.tensor.matmul(ps[:, 0:nn], lhsT=ident,
                                 rhs=src[:, n0:n0 + nn],
                                 start=(k == 0), stop=(k == nf - 1))
            folds[(pair, t)] = ps

        def emit_mini_one(pair, t):
            ps = folds.pop((pair, t))
            slot = acc[:, pair * 10 + t: pair * 10 + t + 1]
            nc.vector.tensor_scalar(ps[:], ps[:], 1.0, None,
                                    op0=mybir.AluOpType.mult,
                                    op1=mybir.AluOpType.add,
                                    accum_out=slot)

        def emit_ared_one(pair, t):
            prod = prods.pop((pair, t))
            slot = acc[:, pair * 10 + t: pair * 10 + t + 1]
            nc.scalar.activation(prod[:], prod[:],
                                 mybir.ActivationFunctionType.Copy,
                                 accum_out=slot)

        def emit_folds(pair):
            # identity-matmul folds [128,SV] -> one PSUM bank (TensorE)
            for t in P_TAPS:
                emit_fold_one(pair, t)

        def emit_gradin(pair):
            half = pair % 2
            G = Gs[pair]
            gin_sb = opool.tile([128, SV], dt.float32, tag="gin",
                                name=f"gin{pair}")
            for (c0, cn) in GI_CHUNKS:
                ps = pspool.tile([128, cn], dt.float32, tag="ps",
                                 padded_shape=[128, 1024],
                                 name=f"ps{pair}_{c0}")
                for t in range(K * K):
                    lhsT = wsb[:, (half * 9 + t) * 128:
                               (half * 9 + t + 1) * 128]
                    off = TAP_OFF[t] + c0
                    for n0 in range(0, cn, 512):
                        nn = min(512, cn - n0)
                        nc.tensor.matmul(
                            ps[:, n0:n0 + nn], lhsT=lhsT,
                            rhs=G[:, off + n0: off + n0 + nn],
                            start=(t == 0), stop=(t == K * K - 1))
                nc.scalar.copy(gin_sb[:, c0:c0 + cn], ps[:])
            nc.sync.dma_start(out=gin_out[pair], in_=gin_sb[:])

        def emit_mults(pair):
            G, X = Gs[pair], Xs[pair]
            for t in range(K * K):
                kind = REDUCE_ENG[t]
                prod = ppool.tile([128, SV], dt.bfloat16,
                                  tag=f"prod_{kind}",
                                  bufs=(10 if kind == 'p' else
                                        8 if kind == 'a' else 2),
                                  name=f"prod{pair}_{t}")
                nc.vector.tensor_mul(prod[:],
                                     X[:, TAP_OFF[t]:TAP_OFF[t] + SV],
                                     G[:, OFF_C:OFF_C + SV])
                slot = acc[:, pair * 10 + t: pair * 10 + t + 1]
                if kind == 'd':
                    nc.vector.tensor_scalar(prod[:], prod[:], 1.0, None,
                                            op0=mybir.AluOpType.mult,
                                            op1=mybir.AluOpType.add,
                                            accum_out=slot)
                elif kind == 't':
                    nc.vector.tensor_add(prod[:, 0:1624], prod[:, 0:1624],
                                         prod[:, 1624:3248])
                    nc.vector.tensor_add(prod[:, 0:812], prod[:, 0:812],
                                         prod[:, 812:1624])
                    nc.vector.tensor_add(prod[:, 0:406], prod[:, 0:406],
                                         prod[:, 406:812])
                    nc.vector.tensor_scalar(prod[:, 0:406], prod[:, 0:406],
                                            1.0, None,
                                            op0=mybir.AluOpType.mult,
                                            op1=mybir.AluOpType.add,
                                            accum_out=slot)
                else:
                    prods[(pair, t)] = prod
            bkind = REDUCE_ENG[9]
            slot = acc[:, pair * 10 + 9: pair * 10 + 10]
            if bkind == 'd':
                bsc = ppool.tile([128, SV], dt.bfloat16, tag="prod_d",
                                 bufs=2, name=f"bias{pair}")
                nc.vector.tensor_scalar(bsc[:], G[:, OFF_C:OFF_C + SV], 1.0,
                                        None, op0=mybir.AluOpType.mult,
                                        op1=mybir.AluOpType.add,
                                        accum_out=slot)
            elif bkind == 't':
                bsc = ppool.tile([128, 1624], dt.bfloat16, tag="prod_t2",
                                 bufs=2, name=f"bias{pair}")
                nc.vector.tensor_add(bsc[:], G[:, OFF_C:OFF_C + 1624],
                                     G[:, OFF_C + 1624:OFF_C + 3248])
                nc.vector.tensor_add(bsc[:, 0:812], bsc[:, 0:812],
                                     bsc[:, 812:1624])
                nc.vector.tensor_add(bsc[:, 0:406], bsc[:, 0:406],
                                     bsc[:, 406:812])
                nc.vector.tensor_scalar(bsc[:, 0:406], bsc[:, 0:406],
                                        1.0, None,
                                        op0=mybir.AluOpType.mult,
                                        op1=mybir.AluOpType.add,
                                        accum_out=slot)
            elif bkind == 'a':
                bsc = ppool.tile([128, SV], dt.bfloat16, tag="prod_a",
                                 bufs=8, name=f"bias{pair}")
                nc.vector.tensor_copy(bsc[:], G[:, OFF_C:OFF_C + SV])
                prods[(pair, 9)] = bsc

        # software pipeline: pair p's folds/minis/ACT-reduces are issued at
        # the head of pair p+1's block so no engine waits on same-block deps
        load(0, split=True)
        nc.sync.dma_start(out=wsb[:, 9 * 128:18 * 128],
                          in_=w_in[:, 9 * 128:18 * 128])
        load(1)
        for pair in range(NPAIR):
            if pair + 2 < NPAIR:
                load(pair + 2)
            if pair == NPAIR - 1:
                # flush completed pairs' accumulators under the last block
                nc.sync.dma_start(out=acc_out[:, 0:60], in_=acc[:, 0:60])
            if pair > 0:
                emit_folds(pair - 1)
                emit_minis(pair - 1)
                emit_act_reduces(pair - 1)
            emit_gradin(pair)
            emit_mults(pair)
        # tail: interleave the last pair's reduces per tap so the
        # pipeline drains with engines chasing each other
        last = NPAIR - 1
        for t in range(10):
            if (last, t) in prods or (REDUCE_ENG[t] == 'p' and t == 9):
                if REDUCE_ENG[t] == 'p':
                    emit_fold_one(last, t)
                    emit_mini_one(last, t)
                elif REDUCE_ENG[t] == 'a':
                    emit_ared_one(last, t)

        nc.sync.dma_start(out=acc_out[:, 60:], in_=acc[:, 60:])

    patched, n_fill = _split_sync_waits(nc.to_json_bytes())
    nc.to_json_bytes = lambda: patched
    return nc


def _host_prep(g_out, inputs, kernels):
    """Transpose to [img, c, padded-spatial] bf16 + build diag weights."""
    bf16 = ml_dtypes.bfloat16

    def to_tiles(a):
        # [B,H,W,C] fp32 -> [B, C, HP*WP(+pad)] bf16 with zero borders
        t = np.zeros((B, C, HP, WP), dtype=bf16)
        t[:, :, 1:H + 1, 1:W + 1] = a.transpose(0, 3, 1, 2)
        t = t.reshape(B, C, S_PAD)
        out = np.zeros((B, C, S_TILE), dtype=bf16)
        out[:, :, :S_PAD] = t
        # -> per-core [NPAIR, 128, S_TILE]; pair = img_local*2 + half
        out = out.reshape(B, 2, 128, S_TILE)
        return [out[c * IMG_PER_CORE:(c + 1) * IMG_PER_CORE].reshape(
            NPAIR, 128, S_TILE) for c in range(NCORES)]

    g_tiles = to_tiles(g_out)
    x_tiles = to_tiles(inputs)

    # rotated weights for grad_in: wr[i,j,c] = kernels[2-i,2-j,c,0]
    wr = kernels[::-1, ::-1, :, 0]          # [3,3,C]
    wsb = np.zeros((128, 19 * 128), dtype=bf16)
    for t in range(K * K):
        i, j = t // K, t % K
        for hhalf in range(2):
            blk = (hhalf * 9 + t) * 128
            wsb[np.arange(128), blk + np.arange(128)] = \
                wr[i, j, hhalf * 128:(hhalf + 1) * 128].astype(bf16)
    # block 18: identity (for TensorE fold-reductions)
    wsb[np.arange(128), 18 * 128 + np.arange(128)] = bf16(1.0)
    return g_tiles, x_tiles, wsb


def _run(in_maps, trace=False, tmpdir=None):
    from concourse.bass_utils import run_bass_kernel_spmd
    if "nc" not in _compiled:
        _compiled["nc"] = _build_kernel()
    nc = _compiled["nc"]
    return run_bass_kernel_spmd(nc, in_maps, list(range(NCORES)),
                                trace=trace, tmpdir=tmpdir)


def kernel(g_out, inputs, kernels, _trace=False, _tmpdir=None):
    g_out = np.asarray(g_out, dtype=np.float32)
    inputs = np.asarray(inputs, dtype=np.float32)
    kernels = np.asarray(kernels, dtype=np.float32)

    g_tiles, x_tiles, wsb = _host_prep(g_out, inputs, kernels)
    in_maps = [{"g": g_tiles[c], "x": x_tiles[c], "wdiag": wsb}
               for c in range(NCORES)]
    res = _run(in_maps, trace=_trace, tmpdir=_tmpdir)
    results = res.results
    kernel._last_exec_time_ns = res.exec_time_ns

    # ---- unshard / assemble ----
    grad_in = np.empty((B, C, H, W), dtype=np.float32)
    gw_acc = np.zeros((128, NPAIR * 10), dtype=np.float64)
    for c in range(NCORES):
        gin = results[c]["gin"]          # [NPAIR, 128, SV] f32
        gin = gin.reshape(NPAIR, 128, H, WP)[:, :, :, :W]
        for p in range(NPAIR):
            img = c * IMG_PER_CORE + p // 2
            hhalf = p % 2
            grad_in[img, hhalf * 128:(hhalf + 1) * 128] = gin[p]
        gw_acc += results[c]["gwacc"].astype(np.float64)
    grad_in = grad_in.transpose(0, 2, 3, 1)

    # gw_acc[c_part, pair*10 + t]: sum over images (pairs with same half)
    gw_acc = gw_acc.reshape(128, IMG_PER_CORE, 2, 10).sum(axis=1)
    grad_wt = np.empty((K, K, C, 1), dtype=np.float32)
    grad_bias = np.empty((C,), dtype=np.float32)
    for hhalf in range(2):
        sl = slice(hhalf * 128, (hhalf + 1) * 128)
        for t in range(K * K):
            grad_wt[t // K, t % K, sl, 0] = gw_acc[:, hhalf, t]
        grad_bias[sl] = gw_acc[:, hhalf, 9]

    return (grad_in, grad_wt.astype(np.float32), grad_bias.astype(np.float32))


# revision 30
# speedup vs baseline: 1.0164x; 1.0075x over previous
"""Trainium2 Bass kernel: BackwardInjectDepthwiseConv2D (B=32,H=W=56,C=256,K=3).

Outputs (grad_in, grad_wt, grad_bias) for a depthwise conv backward:
  grad_in[b,y,x,c]  = sum_{i,j} g_pad[b,y+i,x+j,c] * kernels[2-i,2-j,c,0]
  grad_wt[i,j,c]    = sum_{b,y,x} x_pad[b,y+i,x+j,c] * g_out[b,y,x,c]
  grad_bias[c]      = sum_{b,y,x} g_out[b,y,x,c]

Strategy (8 NeuronCores, pure batch data-parallel, 4 images/core):
 - Host reshapes to channels-on-partitions layout [img, c, 58*58] (zero
   padded borders) in bf16; per core 8 "pairs" = 4 images x 2 channel
   halves of 128.
 - grad_in on TensorE: 9 accumulating diag-matmuls per pair (lhsT =
   diag(w_tap) per channel-half); shifted taps are free-dim slices of
   the padded tile.  PSUM fp32 accumulation, ScalarE evacuates.
 - grad_wt: per tap a bf16 tensor_tensor multiply on VectorE (2x mode);
   the per-partition free-dim reduction of each product is split across
   ScalarE (activation accum_out, 1x) and TensorE (identity-matmul fold
   into one PSUM bank + small VectorE accum) to balance all three
   engines (~85% busy each).
 - grad_bias rides the same reduce path on g directly.
 - Weight/bias grads are per-core partials summed on host; the three
   pipeline stages per pair are software-pipelined one pair apart so
   no engine waits on same-block producers.
 - BIR post-pass splits multi-sem-wait instructions into single-wait
   NoOp prefixes (this walrus build encodes at most one wait/inst).
"""

import sys

for _p in ("/opt/trn_rl_repo",):
    if _p not in sys.path:
        sys.path.insert(0, _p)

import numpy as np
import ml_dtypes
from contextlib import ExitStack

B, H, W, C, K = 32, 56, 56, 256, 3
NCORES = 8
IMG_PER_CORE = B // NCORES          # 4
NPAIR = IMG_PER_CORE * 2            # 4 images x 2 channel halves
HP = H + 2                          # 58 (padded rows)
WP = W + 2                          # 58 (padded cols)
S_PAD = HP * WP                     # 3364
S_TILE = 3368                       # tile free size (tail pad, even)
SV = H * WP                         # 3248: output window (56 rows of 58)
OFF_C = WP + 1                      # 59: offset of g[0,0] inside padded tile
TAP_OFF = [i * WP + j for i in range(K) for j in range(K)]

# chunks of the [128, SV] grad_in PSUM accumulation (bank-aligned fp32)
PS_CHUNKS = [(0, 1536), (1536, 1536), (3072, SV - 3072)]

# which engine reduces each tap's product (9 taps + 1 bias tap)
# 'd' = VectorE tensor_scalar accum (1x, ~3.5us/pair)
# 'a' = ScalarE activation accum   (~3.7us/pair, parallel engine)
# 'p' = TensorE identity-fold to one PSUM bank + VectorE mini-reduce
#       (~1.4us PE + 0.7us DVE per pair)
# 't' = VectorE 2x fold-tree (tensor_tensor adds) + small accum reduce
# 'apapapapap' (5 ScalarE + 5 TensorE-fold) measured best on HW.
import os
REDUCE_ENG = list(os.environ.get("REDUCE_ENG", "apapapapap"))

_compiled = {}


def _split_sync_waits(bir_bytes, maxw=1):
    """walrus in this env encodes at most one sync-wait per instruction;
    split any multi-wait instruction into preceding single-wait Drains."""
    import orjson
    m = orjson.loads(bir_bytes)
    n_new = [0]

    def fix_block(bb):
        insts = bb.get("instructions")
        if not isinstance(insts, list):
            return
        out = []
        for inst in insts:
            si = inst.get("sync_info")
            waits = (si or {}).get("on_wait") or []
            if len(waits) > maxw:
                excess = waits[:-maxw]
                inst["sync_info"]["on_wait"] = waits[-maxw:]
                for k in range(0, len(excess), maxw):
                    n_new[0] += 1
                    filler = {
                        "name": f"I-wsplit-{n_new[0]}",
                        "opcode": "NoOp",
                        "engine": inst["engine"],
                        "ins": [], "outs": [],
                        "sync_info": {"on_update": [],
                                      "on_wait": excess[k:k + maxw]},
                    }
                    if "debug" in inst:
                        filler["debug"] = inst["debug"]
                    out.append(filler)
            out.append(inst)
        bb["instructions"] = out

    def walk(o):
        if isinstance(o, dict):
            if "instructions" in o:
                fix_block(o)
            for v in o.values():
                walk(v)
        elif isinstance(o, list):
            for v in o:
                walk(v)

    walk(m)
    return orjson.dumps(m), n_new[0]


def _build_kernel():
    import concourse.bass as bass
    import concourse.tile as tile
    import concourse.mybir as mybir

    dt = mybir.dt
    nc = bass.Bass("TRN2", target_bir_lowering=False, debug=False,
                   enable_asserts=False, num_devices=NCORES)

    g_in = nc.dram_tensor("g", [NPAIR, 128, S_TILE], dt.bfloat16,
                          kind="ExternalInput").ap()
    x_in = nc.dram_tensor("x", [NPAIR, 128, S_TILE], dt.bfloat16,
                          kind="ExternalInput").ap()
    w_in = nc.dram_tensor("wdiag", [128, 19 * 128], dt.bfloat16,
                          kind="ExternalInput").ap()
    gin_out = nc.dram_tensor("gin", [NPAIR, 128, SV], dt.float32,
                             kind="ExternalOutput").ap()
    acc_out = nc.dram_tensor("gwacc", [128, NPAIR * 10], dt.float32,
                             kind="ExternalOutput").ap()

    P_TAPS = [t for t in range(10) if REDUCE_ENG[t] == 'p']
    # small first chunk so the first ScalarE evac can start early
    GI_CHUNKS = [(0, 512), (512, 1024), (1536, 1024), (2560, SV - 2560)]

    with tile.TileContext(nc) as tc, ExitStack() as ctx:
        wpool = ctx.enter_context(tc.tile_pool(name="w", bufs=1))
        gpool = ctx.enter_context(tc.tile_pool(name="g", bufs=3))
        xpool = ctx.enter_context(tc.tile_pool(name="x", bufs=3))
        ppool = ctx.enter_context(tc.tile_pool(name="prod", bufs=2))
        opool = ctx.enter_context(tc.tile_pool(name="gout", bufs=2))
        apool = ctx.enter_context(tc.tile_pool(name="accp", bufs=1))
        pspool = ctx.enter_context(tc.tile_pool(name="ps", bufs=2,
                                                space="PSUM"))

        wsb = wpool.tile([128, 19 * 128], dt.bfloat16)
        # half-0 weights first, then the head of G0/X0 so all engines
        # start early; half-1 diags and identity load after (see below)
        nc.sync.dma_start(out=wsb[:, 0:9 * 128], in_=w_in[:, 0:9 * 128])
        ident = wsb[:, 18 * 128:19 * 128]

        acc = apool.tile([128, NPAIR * 10], dt.float32)

        Gs, Xs = {}, {}
        prods = {}   # (pair, tap) -> product tile kept for a later stage
        folds = {}   # (pair, tap) -> PSUM fold tile

        def load(pair, split=False):
            G = gpool.tile([128, S_TILE], dt.bfloat16, tag="g",
                           name=f"G{pair}")
            X = xpool.tile([128, S_TILE], dt.bfloat16, tag="x",
                           name=f"X{pair}")
            if split:
                # head of G covers chunks 0-1; X next so VectorE starts
                # early; tail of G only gates chunk 2+
                nc.sync.dma_start(out=G[:, 0:1792], in_=g_in[pair][:, 0:1792])
                nc.sync.dma_start(out=X[:], in_=x_in[pair])
                nc.sync.dma_start(out=G[:, 1792:], in_=g_in[pair][:, 1792:])
                nc.sync.dma_start(out=wsb[:, 18 * 128:],
                                  in_=w_in[:, 18 * 128:])
            else:
                nc.sync.dma_start(out=G[:], in_=g_in[pair])
                nc.sync.dma_start(out=X[:], in_=x_in[pair])
            Gs[pair], Xs[pair] = G, X

        def emit_minis(pair):
            # small PSUM->scalar reduces for last pair's folds (VectorE)
            for t in P_TAPS:
                ps = folds.pop((pair, t))
                slot = acc[:, pair * 10 + t: pair * 10 + t + 1]
                nc.vector.tensor_scalar(ps[:], ps[:], 1.0, None,
                                        op0=mybir.AluOpType.mult,
                                        op1=mybir.AluOpType.add,
                                        accum_out=slot)

        def emit_act_reduces(pair):
            for t in range(10):
                if REDUCE_ENG[t] != 'a':
                    continue
                prod = prods.pop((pair, t))
                slot = acc[:, pair * 10 + t: pair * 10 + t + 1]
                nc.scalar.activation(prod[:], prod[:],
                                     mybir.ActivationFunctionType.Copy,
                                     accum_out=slot)

        def emit_fold_one(pair, t):
            if t < 9:
                src = prods.pop((pair, t))
                src = src[:, 0:SV]
            else:
                src = Gs[pair][:, OFF_C:OFF_C + SV]
            ps = pspool.tile([128, 512], dt.float32, tag="fold",
                             bufs=4, name=f"fold{pair}_{t}")
            nf = (SV + 511) // 512
            for k in range(nf):
                n0 = k * 512
                nn = min(512, SV - n0)
                nc.tensor.matmul(ps[:, 0:nn], lhsT=ident,
                                 rhs=src[:, n0:n0 + nn],
                                 start=(k == 0), stop=(k == nf - 1))
            folds[(pair, t)] = ps

        def emit_mini_one(pair, t):
            ps = folds.pop((pair, t))
            slot = acc[:, pair * 10 + t: pair * 10 + t + 1]
            nc.vector.tensor_scalar(ps[:], ps[:], 1.0, None,
                                    op0=mybir.AluOpType.mult,
                                    op1=mybir.AluOpType.add,
                                    accum_out=slot)

        def emit_ared_one(pair, t):
            prod = prods.pop((pair, t))
            slot = acc[:, pair * 10 + t: pair * 10 + t + 1]
            nc.scalar.activation(prod[:], prod[:],
                                 mybir.ActivationFunctionType.Copy,
                                 accum_out=slot)

        def emit_folds(pair):
            # identity-matmul folds [128,SV] -> one PSUM bank (TensorE)
            for t in P_TAPS:
                emit_fold_one(pair, t)

        def emit_gradin(pair):
            half = pair % 2
            G = Gs[pair]
            gin_sb = opool.tile([128, SV], dt.float32, tag="gin",
                                name=f"gin{pair}")
            for (c0, cn) in GI_CHUNKS:
                ps = pspool.tile([128, cn], dt.float32, tag="ps",
                                 padded_shape=[128, 1024],
                                 name=f"ps{pair}_{c0}")
                for t in range(K * K):
                    lhsT = wsb[:, (half * 9 + t) * 128:
                               (half * 9 + t + 1) * 128]
                    off = TAP_OFF[t] + c0
                    for n0 in range(0, cn, 512):
                        nn = min(512, cn - n0)
                        nc.tensor.matmul(
                            ps[:, n0:n0 + nn], lhsT=lhsT,
                            rhs=G[:, off + n0: off + n0 + nn],
                            start=(t == 0), stop=(t == K * K - 1))
                nc.scalar.copy(gin_sb[:, c0:c0 + cn], ps[:])
            nc.sync.dma_start(out=gin_out[pair], in_=gin_sb[:])

        def emit_mults(pair):
            G, X = Gs[pair], Xs[pair]
            for t in range(K * K):
                kind = REDUCE_ENG[t]
                prod = ppool.tile([128, SV], dt.bfloat16,
                                  tag=f"prod_{kind}",
                                  bufs=(10 if kind == 'p' else
                                        8 if kind == 'a' else 2),
                                  name=f"prod{pair}_{t}")
                nc.vector.tensor_mul(prod[:],
                                     X[:, TAP_OFF[t]:TAP_OFF[t] + SV],
                                     G[:, OFF_C:OFF_C + SV])
                slot = acc[:, pair * 10 + t: pair * 10 + t + 1]
                if kind == 'd':
                    nc.vector.tensor_scalar(prod[:], prod[:], 1.0, None,
                                            op0=mybir.AluOpType.mult,
                                            op1=mybir.AluOpType.add,
                                            accum_out=slot)
                elif kind == 't':
                    nc.vector.tensor_add(prod[:, 0:1624], prod[:, 0:1624],
                                         prod[:, 1624:3248])
                    nc.vector.tensor_add(prod[:, 0:812], prod[:, 0:812],
                                         prod[:, 812:1624])
                    nc.vector.tensor_add(prod[:, 0:406], prod[:, 0:406],
                                         prod[:, 406:812])
                    nc.vector.tensor_scalar(prod[:, 0:406], prod[:, 0:406],
                                            1.0, None,
                                            op0=mybir.AluOpType.mult,
                                            op1=mybir.AluOpType.add,
                                            accum_out=slot)
                else:
                    prods[(pair, t)] = prod
            bkind = REDUCE_ENG[9]
            slot = acc[:, pair * 10 + 9: pair * 10 + 10]
            if bkind == 'd':
                bsc = ppool.tile([128, SV], dt.bfloat16, tag="prod_d",
                                 bufs=2, name=f"bias{pair}")
                nc.vector.tensor_scalar(bsc[:], G[:, OFF_C:OFF_C + SV], 1.0,
                                        None, op0=mybir.AluOpType.mult,
                                        op1=mybir.AluOpType.add,
                                        accum_out=slot)
            elif bkind == 't':
                bsc = ppool.tile([128, 1624], dt.bfloat16, tag="prod_t2",
                                 bufs=2, name=f"bias{pair}")
                nc.vector.tensor_add(bsc[:], G[:, OFF_C:OFF_C + 1624],
                                     G[:, OFF_C + 1624:OFF_C + 3248])
                nc.vector.tensor_add(bsc[:, 0:812], bsc[:, 0:812],
                                     bsc[:, 812:1624])
                nc.vector.tensor_add(bsc[:, 0:406], bsc[:, 0:406],
                                     bsc[:, 406:812])
                nc.vector.tensor_scalar(bsc[:, 0:406], bsc[:, 0:406],
                                        1.0, None,
                                        op0=mybir.AluOpType.mult,
                                        op1=mybir.AluOpType.add,
                                        accum_out=slot)
            elif bkind == 'a':
                bsc = ppool.tile([128, SV], dt.bfloat16, tag="prod_a",
                                 bufs=8, name=f"bias{pair}")
                nc.vector.tensor_copy(bsc[:], G[:, OFF_C:OFF_C + SV])
                prods[(pair, 9)] = bsc

        # software pipeline: pair p's folds/minis/ACT-reduces are issued at
        # the head of pair p+1's block so no engine waits on same-block deps
        load(0, split=True)
        nc.sync.dma_start(out=wsb[:, 9 * 128:18 * 128],
                          in_=w_in[:, 9 * 128:18 * 128])
        load(1)
        for pair in range(NPAIR):
            if pair + 2 < NPAIR:
                load(pair + 2)
            if pair == NPAIR - 1:
                # flush completed pairs' accumulators under the last block
                nc.sync.dma_start(out=acc_out[:, 0:60], in_=acc[:, 0:60])
            if pair > 0:
                emit_folds(pair - 1)
                emit_minis(pair - 1)
                emit_act_reduces(pair - 1)
            emit_gradin(pair)
            emit_mults(pair)
        # tail: interleave the last pair's reduces per tap so the
        # pipeline drains with engines chasing each other
        last = NPAIR - 1
        for t in range(10):
            if (last, t) in prods or (REDUCE_ENG[t] == 'p' and t == 9):
                if REDUCE_ENG[t] == 'p':
                    emit_fold_one(last, t)
                    emit_mini_one(last, t)
                elif REDUCE_ENG[t] == 'a':
                    emit_ared_one(last, t)

        nc.sync.dma_start(out=acc_out[:, 60:], in_=acc[:, 60:])

    patched, n_fill = _split_sync_waits(nc.to_json_bytes())
    nc.to_json_bytes = lambda: patched
    return nc


def _host_prep(g_out, inputs, kernels):
    """Transpose to [img, c, padded-spatial] bf16 + build diag weights."""
    bf16 = ml_dtypes.bfloat16

    def to_tiles(a):
        # [B,H,W,C] fp32 -> [B, C, HP*WP(+pad)] bf16 with zero borders
        t = np.zeros((B, C, HP, WP), dtype=bf16)
        t[:, :, 1:H + 1, 1:W + 1] = a.transpose(0, 3, 1, 2)
        t = t.reshape(B, C, S_PAD)
        out = np.zeros((B, C, S_TILE), dtype=bf16)
        out[:, :, :S_PAD] = t
        # -> per-core [NPAIR, 128, S_TILE]; pair = img_local*2 + half
        out = out.reshape(B, 2, 128, S_TILE)
        return [out[c * IMG_PER_CORE:(c + 1) * IMG_PER_CORE].reshape(
            NPAIR, 128, S_TILE) for c in range(NCORES)]

    g_tiles = to_tiles(g_out)
    x_tiles = to_tiles(inputs)

    # rotated weights for grad_in: wr[i,j,c] = kernels[2-i,2-j,c,0]
    wr = kernels[::-1, ::-1, :, 0]          # [3,3,C]
    wsb = np.zeros((128, 19 * 128), dtype=bf16)
    for t in range(K * K):
        i, j = t // K, t % K
        for hhalf in range(2):
            blk = (hhalf * 9 + t) * 128
            wsb[np.arange(128), blk + np.arange(128)] = \
                wr[i, j, hhalf * 128:(hhalf + 1) * 128].astype(bf16)
    # block 18: identity (for TensorE fold-reductions)
    wsb[np.arange(128), 18 * 128 + np.arange(128)] = bf16(1.0)
    return g_tiles, x_tiles, wsb


def _run(in_maps, trace=False, tmpdir=None):
    from concourse.bass_utils import run_bass_kernel_spmd
    if "nc" not in _compiled:
        _compiled["nc"] = _build_kernel()
    nc = _compiled["nc"]
    return run_bass_kernel_spmd(nc, in_maps, list(range(NCORES)),
                                trace=trace, tmpdir=tmpdir)


def kernel(g_out, inputs, kernels, _trace=False, _tmpdir=None):
    g_out = np.asarray(g_out, dtype=np.float32)
    inputs = np.asarray(inputs, dtype=np.float32)
    kernels = np.asarray(kernels, dtype=np.float32)

    g_tiles, x_tiles, wsb = _host_prep(g_out, inputs, kernels)
    in_maps = [{"g": g_tiles[c], "x": x_tiles[c], "wdiag": wsb}
               for c in range(NCORES)]
    res = _run(in_maps, trace=_trace, tmpdir=_tmpdir)
    results = res.results
    kernel._last_exec_time_ns = res.exec_time_ns

    # ---- unshard / assemble ----
    grad_in = np.empty((B, C, H, W), dtype=np.float32)
    gw_acc = np.zeros((128, NPAIR * 10), dtype=np.float64)
    for c in range(NCORES):
        gin = results[c]["gin"]          # [NPAIR, 128, SV] f32
        gin = gin.reshape(NPAIR, 128, H, WP)[:, :, :, :W]
        for p in range(NPAIR):
            img = c * IMG_PER_CORE + p // 2
            hhalf = p % 2
            grad_in[img, hhalf * 128:(hhalf + 1) * 128] = gin[p]
        gw_acc += results[c]["gwacc"].astype(np.float64)
    grad_in = grad_in.transpose(0, 2, 3, 1)

    # gw_acc[c_part, pair*10 + t]: sum over images (pairs with same half)
    gw_acc = gw_acc.reshape(128, IMG_PER_CORE, 2, 10).sum(axis=1)
    grad_wt = np.empty((K, K, C, 1), dtype=np.float32)
    grad_bias = np.empty((C,), dtype=np.float32)
    for hhalf in range(2):
        sl = slice(hhalf * 128, (hhalf + 1) * 128)
        for t in range(K * K):
            grad_wt[t // K, t % K, sl, 0] = gw_acc[:, hhalf, t]
        grad_bias[sl] = gw_acc[:, hhalf, 9]

    return (grad_in, grad_wt.astype(np.float32), grad_bias.astype(np.float32))
